# revision 7
# baseline (speedup 1.0000x reference)
"""Trainium2 Bass kernel for the CRF loss (nn_CRFModule).

Math: loss = mean_b( logZ_b - gold_b ) for a linear-chain CRF with
B=128, T=1024, K=128 tags, mask all-ones.

Device strategy (8 NeuronCores, SPMD):
  logZ is a chain of T-1 log-space matrix-vector products. In linear space
  each step is  p <- A @ (exp(feat_t) * p)  -- one tiny matmul plus one
  elementwise multiply. The chain is split in half: cores 0-3 run the
  forward half for batch groups 0-3, cores 4-7 run the backward half
  (transposed operator) for the same groups; each core runs one 512-step
  chain over 32 batches in a [K=128 partitions, 32 batch] layout.
  Host stitches the halves:  Z_b = sum_k q511[k,b] * exp(feat[b,512,k]) * p512[k,b].

  Stability: every e-column carries an exp(x-6) bias; every 32 steps the
  state is renormalized by its per-batch column sum (computed via a
  ones-vector matmul, broadcast back via a K=1 matmul, applied two steps
  later off the critical path); log of the sums is accumulated and added
  back on the host.

  The gold (numerator) score is a sparse gather-sum -- O(B*T) -- done on
  host in numpy; the O(B*T*K^2) partition function runs on device.

Self-contained: hardcodes B=128, T=1024, K=128, 8 cores.
"""

import sys

import numpy as np

sys.path.insert(0, "/opt/trn_rl_repo")

B, T, K = 128, 1024, 128
NCORES = 8
BPC = B // 4          # batches per core-pair (32)
STEPS = 512           # chain steps per core
NCHUNK = STEPS // 4   # 128 e-stream chunks of [128, 128] (4 timesteps x 32 batches)
BIAS = 6.0
RENORM = tuple(range(32, 481, 32))
APPLY = tuple(s + 2 for s in RENORM)

_CACHE = {}


def _build_program():
    import concourse.bass as bass
    import concourse.mybir as mybir
    from concourse import bacc
    from concourse.tile import TileContext

    f32 = mybir.dt.float32
    bf16 = mybir.dt.bfloat16

    nc = bacc.Bacc("TRN2", debug=False, target_bir_lowering=False)

    est_d = nc.declare_dram_parameter("estream", [NCHUNK, K, K], f32, isOutput=False)
    w_d = nc.declare_dram_parameter("w_lhsT", [K, K], bf16, isOutput=False)
    onec_d = nc.declare_dram_parameter("ones_col", [K, 1], bf16, isOutput=False)
    oner_d = nc.declare_dram_parameter("ones_row", [1, K], f32, isOutput=False)
    st511_d = nc.declare_dram_parameter("st511", [K, BPC], f32, isOutput=True)
    st512_d = nc.declare_dram_parameter("st512", [K, BPC], f32, isOutput=True)
    logacc_d = nc.declare_dram_parameter("logacc", [1, BPC], f32, isOutput=True)

    with TileContext(nc) as tc:
        with (
            tc.tile_pool(name="const", bufs=1) as constp,
            tc.tile_pool(name="raw", bufs=6) as rawp,
            tc.tile_pool(name="eb", bufs=10) as ebp,
            tc.tile_pool(name="stage", bufs=3) as stagep,
            tc.tile_pool(name="tmp", bufs=2) as tmpp,
            tc.tile_pool(name="sc", bufs=2) as scp,
            tc.tile_pool(name="pp", bufs=3, space=bass.MemorySpace.PSUM) as ppp,
            tc.tile_pool(name="sps", bufs=1, space=bass.MemorySpace.PSUM) as spsp,
            tc.tile_pool(name="bsp", bufs=1, space=bass.MemorySpace.PSUM) as bsp,
        ):
            w_sb = constp.tile([K, K], bf16)
            nc.sync.dma_start(out=w_sb[:], in_=w_d[:])
            onec = constp.tile([K, 1], bf16)
            nc.sync.dma_start(out=onec[:], in_=onec_d[:])
            oner = constp.tile([1, K], f32)
            nc.sync.dma_start(out=oner[:], in_=oner_d[:])
            logacc = constp.tile([1, BPC], f32)
            nc.vector.memset(logacc[:], 0.0)
            negbias = constp.tile([K, 1], f32)
            nc.vector.memset(negbias[:], -BIAS)

            ebs = [None] * NCHUNK
            p_prev = None
            bs_cur = None
            for c in range(NCHUNK):
                raw = rawp.tile([K, K], f32)
                nc.sync.dma_start(out=raw[:], in_=est_d[c])
                eb = ebp.tile([K, K], bf16)
                nc.scalar.activation(
                    eb[:], raw[:], mybir.ActivationFunctionType.Exp, bias=negbias[:]
                )
                ebs[c] = eb

                for tt in range(4):
                    s = 4 * c + tt + 1  # step index, 1..512
                    ecol = ebs[c][:, tt::4]  # [K, 32]
                    if s == 1:
                        rhs = ecol
                    elif s in APPLY:
                        tmp = tmpp.tile([K, BPC], bf16)
                        nc.vector.tensor_mul(tmp[:], p_prev[:], ecol)
                        stage = stagep.tile([K, BPC], bf16)
                        nc.vector.tensor_mul(stage[:], tmp[:], bs_cur[:])
                        rhs = stage[:]
                    else:
                        stage = stagep.tile([K, BPC], bf16)
                        nc.vector.tensor_mul(stage[:], p_prev[:], ecol)
                        rhs = stage[:]

                    p = ppp.tile([K, BPC], f32)
                    nc.tensor.matmul(p[:], w_sb[:], rhs)

                    if s in RENORM:
                        sps = spsp.tile([1, BPC], f32)
                        nc.tensor.matmul(sps[:], onec[:], rhs)
                        rs = scp.tile([1, BPC], f32, tag="rs")
                        nc.vector.reciprocal(rs[:], sps[:])
                        bs_cur = bsp.tile([K, BPC], f32)
                        nc.tensor.matmul(bs_cur[:], oner[:], rs[:])
                        lns = scp.tile([1, BPC], f32, tag="lns")
                        nc.scalar.activation(
                            lns[:], sps[:], mybir.ActivationFunctionType.Ln
                        )
                        nc.vector.tensor_add(logacc[:], logacc[:], lns[:])

                    if s in (511, 512):
                        out_sb = scp.tile([K, BPC], f32, tag=f"out{s}")
                        nc.vector.tensor_copy(out_sb[:], p[:])
                        nc.sync.dma_start(
                            out=(st511_d if s == 511 else st512_d)[:], in_=out_sb[:]
                        )
                    p_prev = p

            nc.sync.dma_start(out=logacc_d[:], in_=logacc[:])

    nc.compile()
    return nc


def _get_program():
    if "nc" not in _CACHE:
        _CACHE["nc"] = _build_program()
    return _CACHE["nc"]


def _host_inputs(feats, transitions, start_transitions, stop_transitions):
    """Build the 8 per-core input dicts."""
    f32 = np.float32
    feats = np.asarray(feats, f32)
    start = np.asarray(start_transitions, f32)
    stop = np.asarray(stop_transitions, f32)
    A = np.exp(np.asarray(transitions, f32))

    import ml_dtypes

    bf16 = ml_dtypes.bfloat16
    w_fwd = np.ascontiguousarray(A.T).astype(bf16)
    w_bwd = np.ascontiguousarray(A).astype(bf16)
    ones_col = np.ones((K, 1), bf16)
    ones_row = np.ones((1, K), f32)

    in_maps = []
    for core in range(NCORES):
        c = core % 4
        bsl = slice(BPC * c, BPC * (c + 1))
        E = np.empty((STEPS, BPC, K), f32)
        if core < 4:
            E[0] = feats[bsl, 0, :] + start[None, :]
            E[1:STEPS] = feats[bsl, 1:STEPS, :].transpose(1, 0, 2)
        else:
            E[0] = feats[bsl, T - 1, :] + stop[None, :]
            E[1:STEPS - 1] = feats[bsl, np.arange(T - 2, STEPS, -1), :].transpose(1, 0, 2)
            E[STEPS - 1] = BIAS  # dummy column: exp(6-6) = 1
        E4 = E.reshape(NCHUNK, 4, BPC, K)
        est = np.ascontiguousarray(E4.transpose(0, 3, 2, 1).reshape(NCHUNK, K, K))
        in_maps.append(
            {
                "estream": est,
                "w_lhsT": w_fwd if core < 4 else w_bwd,
                "ones_col": ones_col,
                "ones_row": ones_row,
            }
        )
    return in_maps


def _host_gold(feats, transitions, start, stop, tags, mask):
    b = mask.shape[0]
    tags = np.asarray(tags).astype(np.int64)
    feats = np.asarray(feats, np.float32)
    mask = np.asarray(mask, bool)
    trans_score = transitions[tags[:, 1:], tags[:, :-1]]
    emit = np.take_along_axis(feats, tags[:, :, None], axis=2)[..., 0]
    score = np.where(mask[:, 1:], trans_score + emit[:, 1:], 0.0).sum(-1, dtype=np.float64)
    score = score + emit[:, 0] + start[tags[:, 0]]
    last_idx = mask.astype(np.int32).sum(-1) - 1
    last_tags = tags[np.arange(b), last_idx]
    return score + stop[last_tags]


def _combine(results, feats):
    logZ = np.zeros(B, np.float64)
    for c in range(4):
        bsl = slice(BPC * c, BPC * (c + 1))
        p512 = results[c]["st512"].astype(np.float64)       # [K, 32]
        laf = results[c]["logacc"][0].astype(np.float64)    # [32]
        q511 = results[c + 4]["st511"].astype(np.float64)   # [K, 32]
        lab = results[c + 4]["logacc"][0].astype(np.float64)
        e512 = np.exp(np.asarray(feats[bsl, 512, :], np.float64))  # [32, K]
        dot = (p512 * e512.T * q511).sum(0)
        logZ[bsl] = np.log(dot) + laf + lab + BIAS * T - BIAS
    return logZ


def run_device(in_maps):
    from concourse.bass_utils import run_bass_kernel_spmd

    nc = _get_program()
    res = run_bass_kernel_spmd(nc, in_maps, list(range(NCORES)))
    return res.results


def kernel(feats, transitions, start_transitions, stop_transitions, tags, mask):
    feats = np.asarray(feats)
    transitions = np.asarray(transitions, np.float32)
    start = np.asarray(start_transitions, np.float32)
    stop = np.asarray(stop_transitions, np.float32)

    in_maps = _host_inputs(feats, transitions, start, stop)
    results = run_device(in_maps)
    logZ = _combine(results, np.asarray(feats, np.float32))
    gold = _host_gold(feats, transitions, start, stop, tags, mask)
    loss = (logZ - gold).mean()
    return np.array(loss, dtype=np.float32)


# revision 24
# speedup vs baseline: 7939.8576x; 7939.8576x over previous
"""Trainium2 Bass kernel for the CRF loss (nn_CRFModule).

Math: loss = mean_b( logZ_b - gold_b ) for a linear-chain CRF with
B=128, T=1024, K=128 tags, mask all-ones.

Device strategy (8 NeuronCores, SPMD):
  logZ is a chain of T-1 log-space matrix-vector products. In linear space
  each step is  p <- A @ (exp(feat_t) * p)  -- one tiny matmul plus one
  elementwise multiply. The chain is split in half: cores 0-3 run the
  forward half for batch groups 0-3, cores 4-7 run the backward half
  (transposed operator) for the same groups; each core runs one 512-step
  chain over 32 batches in a [K=128 partitions, 32 batch] layout.
  Host stitches the halves:  Z_b = sum_k q511[k,b] * exp(feat[b,512,k]) * p512[k,b].

  Stability: every e-column carries an exp(x-6) bias; every 64 steps the
  state is renormalized by its per-batch column sum (ones-vector matmul ->
  reciprocal -> K=1 broadcast matmul -> pre-scaled into a later e-column).
  Each sub-op is deferred several steps after its input is produced so the
  in-order engine sequencers never stall the chain on a renorm dependency;
  the scaling lands 12 steps after the sum with exact ln-compensation
  accumulated and added back on the host.

  The gold (numerator) score is a sparse gather-sum -- O(B*T) -- done on
  host in numpy; the O(B*T*K^2) partition function runs on device.

Self-contained: hardcodes B=128, T=1024, K=128, 8 cores.
"""

import sys

import numpy as np

sys.path.insert(0, "/opt/trn_rl_repo")

B, T, K = 128, 1024, 128
NCORES = 8
BPC = B // 4          # batches per core-pair (32)
STEPS = 512           # chain steps per core
NCHUNK = STEPS // 4   # 128 e-stream chunks of [128, 128] (4 timesteps x 32 batches)
BIAS = 6.0
RENORM = tuple(range(64, 481, 64))
APPLY = tuple(s + 12 for s in RENORM)

_CACHE = {}


def _build_program():
    import concourse.bass as bass
    import concourse.mybir as mybir
    from concourse import bacc
    from concourse.tile import TileContext

    f32 = mybir.dt.float32
    bf16 = mybir.dt.bfloat16

    nc = bacc.Bacc("TRN2", debug=False, target_bir_lowering=False)

    est_d = nc.declare_dram_parameter("estream", [NCHUNK, K, K], bf16, isOutput=False)
    w_d = nc.declare_dram_parameter("w_lhsT", [K, K], bf16, isOutput=False)
    onec_d = nc.declare_dram_parameter("ones_col", [K, 1], bf16, isOutput=False)
    oner_d = nc.declare_dram_parameter("ones_row", [1, K], f32, isOutput=False)
    st511_d = nc.declare_dram_parameter("st511", [K, BPC], f32, isOutput=True)
    st512_d = nc.declare_dram_parameter("st512", [K, BPC], f32, isOutput=True)
    logacc_d = nc.declare_dram_parameter("logacc", [1, BPC], f32, isOutput=True)

    with TileContext(nc) as tc:
        with (
            tc.tile_pool(name="const", bufs=1) as constp,
            tc.tile_pool(name="raw", bufs=6) as rawp,
            tc.tile_pool(name="eb", bufs=10) as ebp,
            tc.tile_pool(name="stage", bufs=8) as stagep,
            tc.tile_pool(name="tmp", bufs=2) as tmpp,
            tc.tile_pool(name="sc", bufs=2) as scp,
            tc.tile_pool(name="pp", bufs=6, space=bass.MemorySpace.PSUM) as ppp,
            tc.tile_pool(name="sps", bufs=1, space=bass.MemorySpace.PSUM) as spsp,
            tc.tile_pool(name="bsp", bufs=1, space=bass.MemorySpace.PSUM) as bsp,
        ):
            w_sb = constp.tile([K, K], bf16)
            nc.sync.dma_start(out=w_sb[:], in_=w_d[:])
            onec = constp.tile([K, 1], bf16)
            nc.sync.dma_start(out=onec[:], in_=onec_d[:])
            oner = constp.tile([1, K], f32)
            nc.sync.dma_start(out=oner[:], in_=oner_d[:])
            logacc = constp.tile([1, BPC], f32)
            nc.vector.memset(logacc[:], 0.0)
            negbias = constp.tile([K, 1], f32)
            nc.vector.memset(negbias[:], -BIAS)

            ebs = [None] * NCHUNK
            p_prev = None
            rn = {}        # live renorm tiles: sps, rs, bs, esc
            deferred = {}  # step -> list of emit callbacks (run after that
                           # step's chain ops so in-order seqs never stall)
            for c in range(NCHUNK):
                raw = rawp.tile([K, K], bf16)
                nc.sync.dma_start(out=raw[:], in_=est_d[c])
                eb = ebp.tile([K, K], bf16)
                nc.scalar.activation(
                    eb[:], raw[:], mybir.ActivationFunctionType.Exp, bias=negbias[:]
                )
                ebs[c] = eb

                for tt in range(4):
                    s = 4 * c + tt + 1  # step index, 1..512
                    ecol = ebs[c][:, tt * BPC:(tt + 1) * BPC]  # [K, 32] packed
                    if s in APPLY:
                        ecol = rn.pop("esc")[:]  # pre-scaled by renorm factor
                    if s == 1:
                        rhs = ecol
                    else:
                        stage = stagep.tile([K, BPC], bf16)
                        nc.vector.tensor_mul(stage[:], p_prev[:], ecol)
                        rhs = stage[:]

                    p = ppp.tile([K, BPC], f32)
                    nc.tensor.matmul(p[:], w_sb[:], rhs)

                    if s in RENORM:
                        # column-sum now (input is hot); defer the dependent
                        # ops so the in-order PE/DVE seqs reach them only
                        # after their inputs are long ready.
                        sps = spsp.tile([1, BPC], f32)
                        nc.tensor.matmul(sps[:], onec[:], rhs)
                        rn["sps"] = sps

                        def d_recip():
                            rn["rs"] = scp.tile([1, BPC], f32, tag="rs", name="rs")
                            nc.vector.reciprocal(rn["rs"][:], rn["sps"][:])

                        def d_bcast():
                            rn["bs"] = bsp.tile([K, BPC], f32, name="bs")
                            nc.tensor.matmul(rn["bs"][:], oner[:], rn["rs"][:])

                        def d_esc(col=4 * c + tt + 12):
                            # pre-scale the e-column consumed at step s+6
                            ec = ebs[col // 4][:, (col % 4) * BPC:(col % 4 + 1) * BPC]
                            rn["esc"] = tmpp.tile([K, BPC], bf16, tag="esc", name="esc")
                            nc.vector.tensor_mul(rn["esc"][:], ec, rn["bs"][:])

                        def d_log():
                            lns = scp.tile([1, BPC], f32, tag="lns")
                            nc.scalar.activation(
                                lns[:], rn["sps"][:], mybir.ActivationFunctionType.Ln
                            )
                            nc.vector.tensor_add(logacc[:], logacc[:], lns[:])

                        deferred.setdefault(s + 3, []).append(d_recip)
                        deferred.setdefault(s + 6, []).append(d_bcast)
                        deferred.setdefault(s + 9, []).append(d_esc)
                        deferred.setdefault(s + 14, []).append(d_log)

                    if s in (511, 512):
                        out_sb = scp.tile([K, BPC], f32, tag=f"out{s}")
                        nc.vector.tensor_copy(out_sb[:], p[:])
                        nc.sync.dma_start(
                            out=(st511_d if s == 511 else st512_d)[:], in_=out_sb[:]
                        )
                    p_prev = p
                    for fn in deferred.pop(s, []):
                        fn()

            nc.sync.dma_start(out=logacc_d[:], in_=logacc[:])

    nc.compile()
    return nc


def _get_program():
    if "nc" not in _CACHE:
        _CACHE["nc"] = _build_program()
    return _CACHE["nc"]


def _host_inputs(feats, transitions, start_transitions, stop_transitions):
    """Build the 8 per-core input dicts."""
    f32 = np.float32
    feats = np.asarray(feats, f32)
    start = np.asarray(start_transitions, f32)
    stop = np.asarray(stop_transitions, f32)
    A = np.exp(np.asarray(transitions, f32))

    import ml_dtypes

    bf16 = ml_dtypes.bfloat16
    w_fwd = np.ascontiguousarray(A.T).astype(bf16)
    w_bwd = np.ascontiguousarray(A).astype(bf16)
    ones_col = np.ones((K, 1), bf16)
    ones_row = np.ones((1, K), f32)

    in_maps = []
    for core in range(NCORES):
        c = core % 4
        bsl = slice(BPC * c, BPC * (c + 1))
        E = np.empty((STEPS, BPC, K), f32)
        if core < 4:
            E[0] = feats[bsl, 0, :] + start[None, :]
            E[1:STEPS] = feats[bsl, 1:STEPS, :].transpose(1, 0, 2)
        else:
            E[0] = feats[bsl, T - 1, :] + stop[None, :]
            E[1:STEPS - 1] = feats[bsl, np.arange(T - 2, STEPS, -1), :].transpose(1, 0, 2)
            E[STEPS - 1] = BIAS  # dummy column: exp(6-6) = 1
        E4 = E.reshape(NCHUNK, 4, BPC, K)
        # chunk layout [k, tt*BPC + b]: ecol slices are contiguous
        est = np.ascontiguousarray(
            E4.transpose(0, 3, 1, 2).reshape(NCHUNK, K, K)).astype(bf16)
        in_maps.append(
            {
                "estream": est,
                "w_lhsT": w_fwd if core < 4 else w_bwd,
                "ones_col": ones_col,
                "ones_row": ones_row,
            }
        )
    return in_maps


def _host_gold(feats, transitions, start, stop, tags, mask):
    b = mask.shape[0]
    tags = np.asarray(tags).astype(np.int64)
    feats = np.asarray(feats, np.float32)
    mask = np.asarray(mask, bool)
    trans_score = transitions[tags[:, 1:], tags[:, :-1]]
    emit = np.take_along_axis(feats, tags[:, :, None], axis=2)[..., 0]
    score = np.where(mask[:, 1:], trans_score + emit[:, 1:], 0.0).sum(-1, dtype=np.float64)
    score = score + emit[:, 0] + start[tags[:, 0]]
    last_idx = mask.astype(np.int32).sum(-1) - 1
    last_tags = tags[np.arange(b), last_idx]
    return score + stop[last_tags]


def _combine(results, feats):
    logZ = np.zeros(B, np.float64)
    for c in range(4):
        bsl = slice(BPC * c, BPC * (c + 1))
        p512 = results[c]["st512"].astype(np.float64)       # [K, 32]
        laf = results[c]["logacc"][0].astype(np.float64)    # [32]
        q511 = results[c + 4]["st511"].astype(np.float64)   # [K, 32]
        lab = results[c + 4]["logacc"][0].astype(np.float64)
        e512 = np.exp(np.asarray(feats[bsl, 512, :], np.float64))  # [32, K]
        dot = (p512 * e512.T * q511).sum(0)
        logZ[bsl] = np.log(dot) + laf + lab + BIAS * T - BIAS
    return logZ


def run_device(in_maps):
    from concourse.bass_utils import run_bass_kernel_spmd

    nc = _get_program()
    res = run_bass_kernel_spmd(nc, in_maps, list(range(NCORES)))
    return res.results


def kernel(feats, transitions, start_transitions, stop_transitions, tags, mask):
    feats = np.asarray(feats)
    transitions = np.asarray(transitions, np.float32)
    start = np.asarray(start_transitions, np.float32)
    stop = np.asarray(stop_transitions, np.float32)

    in_maps = _host_inputs(feats, transitions, start, stop)
    results = run_device(in_maps)
    logZ = _combine(results, np.asarray(feats, np.float32))
    gold = _host_gold(feats, transitions, start, stop, tags, mask)
    loss = (logZ - gold).mean()
    return np.array(loss, dtype=np.float32)



# revision 26
# speedup vs baseline: 8102.7102x; 1.0205x over previous
"""Trainium2 Bass kernel for the CRF loss (nn_CRFModule).

Math: loss = mean_b( logZ_b - gold_b ) for a linear-chain CRF with
B=128, T=1024, K=128 tags, mask all-ones.

Device strategy (8 NeuronCores, SPMD):
  logZ is a chain of T-1 log-space matrix-vector products. In linear space
  each step is  p <- A @ (exp(feat_t) * p)  -- one tiny matmul plus one
  elementwise multiply. The chain is split in half: cores 0-3 run the
  forward half for batch groups 0-3, cores 4-7 run the backward half
  (transposed operator) for the same groups; each core runs one 512-step
  chain over 32 batches in a [K=128 partitions, 32 batch] layout.
  Host stitches the halves:  Z_b = sum_k q511[k,b] * exp(feat[b,512,k]) * p512[k,b].

  Stability: every e-column carries an exp(x-6) bias; every 64 steps the
  state is renormalized by its per-batch column sum (ones-vector matmul ->
  reciprocal -> K=1 broadcast matmul -> pre-scaled into a later e-column).
  Each sub-op is deferred several steps after its input is produced so the
  in-order engine sequencers never stall the chain on a renorm dependency;
  the scaling lands 12 steps after the sum with exact ln-compensation
  accumulated and added back on the host.

  The gold (numerator) score is a sparse gather-sum -- O(B*T) -- done on
  host in numpy; the O(B*T*K^2) partition function runs on device.

Self-contained: hardcodes B=128, T=1024, K=128, 8 cores.
"""

import sys

import numpy as np

sys.path.insert(0, "/opt/trn_rl_repo")

B, T, K = 128, 1024, 128
NCORES = 8
BPC = B // 4          # batches per core-pair (32)
STEPS = 512           # chain steps per core
NCHUNK = STEPS // 4   # 128 e-stream chunks of [128, 128] (4 timesteps x 32 batches)
BIAS = 6.0
RENORM = tuple(range(64, 481, 64))
APPLY = tuple(s + 12 for s in RENORM)

_CACHE = {}


def _build_program():
    import concourse.bass as bass
    import concourse.mybir as mybir
    from concourse import bacc
    from concourse.tile import TileContext

    f32 = mybir.dt.float32
    bf16 = mybir.dt.bfloat16

    nc = bacc.Bacc("TRN2", debug=False, target_bir_lowering=False)

    est_d = nc.declare_dram_parameter("estream", [NCHUNK, K, K], bf16, isOutput=False)
    w_d = nc.declare_dram_parameter("w_lhsT", [K, K], bf16, isOutput=False)
    onec_d = nc.declare_dram_parameter("ones_col", [K, 1], bf16, isOutput=False)
    oner_d = nc.declare_dram_parameter("ones_row", [1, K], f32, isOutput=False)
    st511_d = nc.declare_dram_parameter("st511", [K, BPC], f32, isOutput=True)
    st512_d = nc.declare_dram_parameter("st512", [K, BPC], f32, isOutput=True)
    logacc_d = nc.declare_dram_parameter("logacc", [1, BPC], f32, isOutput=True)

    with TileContext(nc) as tc:
        with (
            tc.tile_pool(name="const", bufs=1) as constp,
            tc.tile_pool(name="raw", bufs=6) as rawp,
            tc.tile_pool(name="eb", bufs=10) as ebp,
            tc.tile_pool(name="stage", bufs=8) as stagep,
            tc.tile_pool(name="tmp", bufs=2) as tmpp,
            tc.tile_pool(name="sc", bufs=2) as scp,
            tc.tile_pool(name="pp", bufs=3, space=bass.MemorySpace.PSUM) as ppp,
            tc.tile_pool(name="sps", bufs=1, space=bass.MemorySpace.PSUM) as spsp,
            tc.tile_pool(name="bsp", bufs=1, space=bass.MemorySpace.PSUM) as bsp,
        ):
            w_sb = constp.tile([K, K], bf16)
            nc.sync.dma_start(out=w_sb[:], in_=w_d[:])
            onec = constp.tile([K, 1], bf16)
            nc.sync.dma_start(out=onec[:], in_=onec_d[:])
            oner = constp.tile([1, K], f32)
            nc.sync.dma_start(out=oner[:], in_=oner_d[:])
            logacc = constp.tile([1, BPC], f32)
            nc.vector.memset(logacc[:], 0.0)
            negbias = constp.tile([K, 1], f32)
            nc.vector.memset(negbias[:], -BIAS)

            ebs = [None] * NCHUNK
            HB = BPC // 2  # 16-column halves: two independent chains
            p_prev = [None, None]
            rn = {}        # live renorm tiles
            deferred = {}  # step -> list of emit callbacks (run after that
                           # step's chain ops so in-order seqs never stall)
            for c in range(NCHUNK):
                raw = rawp.tile([K, K], bf16)
                nc.sync.dma_start(out=raw[:], in_=est_d[c])
                eb = ebp.tile([K, K], bf16)
                nc.scalar.activation(
                    eb[:], raw[:], mybir.ActivationFunctionType.Exp, bias=negbias[:]
                )
                ebs[c] = eb

                for tt in range(4):
                    s = 4 * c + tt + 1  # step index, 1..512
                    for h in range(2):
                        lo = tt * BPC + h * HB
                        if s in APPLY:
                            ecol = rn["esc"][:, h * HB:(h + 1) * HB]
                        else:
                            ecol = ebs[c][:, lo:lo + HB]  # [K, 16] packed
                        if s == 1:
                            rhs = ecol
                        else:
                            stage = stagep.tile([K, HB], bf16, tag=f"st{h}",
                                                name=f"st{h}")
                            nc.vector.tensor_mul(stage[:], p_prev[h][:], ecol)
                            rhs = stage[:]

                        p = ppp.tile([K, HB], f32, tag=f"p{h}", name=f"p{h}",
                                     bufs=3)
                        nc.tensor.matmul(p[:], w_sb[:], rhs)

                        if s in RENORM:
                            if h == 0:
                                rn["sps"] = spsp.tile([1, BPC], f32, name="sps")
                            nc.tensor.matmul(
                                rn["sps"][:, h * HB:(h + 1) * HB], onec[:], rhs)

                        if s in (511, 512):
                            out_sb = scp.tile([K, HB], f32, tag=f"out{s}{h}")
                            nc.vector.tensor_copy(out_sb[:], p[:])
                            od = st511_d if s == 511 else st512_d
                            nc.sync.dma_start(
                                out=od[:, h * HB:(h + 1) * HB], in_=out_sb[:])
                        p_prev[h] = p

                    if s in RENORM:
                        def d_recip():
                            rn["rs"] = scp.tile([1, BPC], f32, tag="rs", name="rs")
                            nc.vector.reciprocal(rn["rs"][:], rn["sps"][:])

                        def d_bcast():
                            rn["bs"] = bsp.tile([K, BPC], f32, name="bs")
                            nc.tensor.matmul(rn["bs"][:], oner[:], rn["rs"][:])

                        def d_esc(col=4 * c + tt + 12):
                            ec = ebs[col // 4][:, (col % 4) * BPC:
                                               (col % 4 + 1) * BPC]
                            rn["esc"] = tmpp.tile([K, BPC], bf16, tag="esc",
                                                  name="esc")
                            nc.vector.tensor_mul(rn["esc"][:], ec, rn["bs"][:])

                        def d_log():
                            lns = scp.tile([1, BPC], f32, tag="lns")
                            nc.scalar.activation(
                                lns[:], rn["sps"][:],
                                mybir.ActivationFunctionType.Ln)
                            nc.vector.tensor_add(logacc[:], logacc[:], lns[:])

                        deferred.setdefault(s + 3, []).append(d_recip)
                        deferred.setdefault(s + 6, []).append(d_bcast)
                        deferred.setdefault(s + 9, []).append(d_esc)
                        deferred.setdefault(s + 14, []).append(d_log)

                    for fn in deferred.pop(s, []):
                        fn()

            nc.sync.dma_start(out=logacc_d[:], in_=logacc[:])

    nc.compile()
    return nc


def _get_program():
    if "nc" not in _CACHE:
        _CACHE["nc"] = _build_program()
    return _CACHE["nc"]


def _host_inputs(feats, transitions, start_transitions, stop_transitions):
    """Build the 8 per-core input dicts."""
    f32 = np.float32
    feats = np.asarray(feats, f32)
    start = np.asarray(start_transitions, f32)
    stop = np.asarray(stop_transitions, f32)
    A = np.exp(np.asarray(transitions, f32))

    import ml_dtypes

    bf16 = ml_dtypes.bfloat16
    w_fwd = np.ascontiguousarray(A.T).astype(bf16)
    w_bwd = np.ascontiguousarray(A).astype(bf16)
    ones_col = np.ones((K, 1), bf16)
    ones_row = np.ones((1, K), f32)

    in_maps = []
    for core in range(NCORES):
        c = core % 4
        bsl = slice(BPC * c, BPC * (c + 1))
        E = np.empty((STEPS, BPC, K), f32)
        if core < 4:
            E[0] = feats[bsl, 0, :] + start[None, :]
            E[1:STEPS] = feats[bsl, 1:STEPS, :].transpose(1, 0, 2)
        else:
            E[0] = feats[bsl, T - 1, :] + stop[None, :]
            E[1:STEPS - 1] = feats[bsl, np.arange(T - 2, STEPS, -1), :].transpose(1, 0, 2)
            E[STEPS - 1] = BIAS  # dummy column: exp(6-6) = 1
        E4 = E.reshape(NCHUNK, 4, BPC, K)
        # chunk layout [k, tt*BPC + b]: ecol slices are contiguous
        est = np.ascontiguousarray(
            E4.transpose(0, 3, 1, 2).reshape(NCHUNK, K, K)).astype(bf16)
        in_maps.append(
            {
                "estream": est,
                "w_lhsT": w_fwd if core < 4 else w_bwd,
                "ones_col": ones_col,
                "ones_row": ones_row,
            }
        )
    return in_maps


def _host_gold(feats, transitions, start, stop, tags, mask):
    b = mask.shape[0]
    tags = np.asarray(tags).astype(np.int64)
    feats = np.asarray(feats, np.float32)
    mask = np.asarray(mask, bool)
    trans_score = transitions[tags[:, 1:], tags[:, :-1]]
    emit = np.take_along_axis(feats, tags[:, :, None], axis=2)[..., 0]
    score = np.where(mask[:, 1:], trans_score + emit[:, 1:], 0.0).sum(-1, dtype=np.float64)
    score = score + emit[:, 0] + start[tags[:, 0]]
    last_idx = mask.astype(np.int32).sum(-1) - 1
    last_tags = tags[np.arange(b), last_idx]
    return score + stop[last_tags]


def _combine(results, feats):
    logZ = np.zeros(B, np.float64)
    for c in range(4):
        bsl = slice(BPC * c, BPC * (c + 1))
        p512 = results[c]["st512"].astype(np.float64)       # [K, 32]
        laf = results[c]["logacc"][0].astype(np.float64)    # [32]
        q511 = results[c + 4]["st511"].astype(np.float64)   # [K, 32]
        lab = results[c + 4]["logacc"][0].astype(np.float64)
        e512 = np.exp(np.asarray(feats[bsl, 512, :], np.float64))  # [32, K]
        dot = (p512 * e512.T * q511).sum(0)
        logZ[bsl] = np.log(dot) + laf + lab + BIAS * T - BIAS
    return logZ


def run_device(in_maps):
    from concourse.bass_utils import run_bass_kernel_spmd

    nc = _get_program()
    res = run_bass_kernel_spmd(nc, in_maps, list(range(NCORES)))
    return res.results


def kernel(feats, transitions, start_transitions, stop_transitions, tags, mask):
    feats = np.asarray(feats)
    transitions = np.asarray(transitions, np.float32)
    start = np.asarray(start_transitions, np.float32)
    stop = np.asarray(stop_transitions, np.float32)

    in_maps = _host_inputs(feats, transitions, start, stop)
    results = run_device(in_maps)
    logZ = _combine(results, np.asarray(feats, np.float32))
    gold = _host_gold(feats, transitions, start, stop, tags, mask)
    loss = (logZ - gold).mean()
    return np.array(loss, dtype=np.float32)

